# revision 1
# baseline (speedup 1.0000x reference)
"""Per-channel batched Linear (OD matrix) Trainium2 Bass kernel.

Computes out[b,o,c] = sum_t x[b,t,c] * W[c,o,t] + bias[c,o] for
x [128,48,64,64] -> [128,48,4096], W [4096,48,48], bias [4096,48].

Strategy (8 NeuronCores, channel-parallel, 512 channels/core):
  - x^T loaded HBM->SBUF with strided APs: partitions = (j2, t48) rows
    {0-47, 64-111}, free = (b, g) with 128-channel innermost runs (512B).
  - ACT casts x to bf16 with (b,g)->(g,b) permute so each channel's
    lhsT [49, 128] is contiguous (FWL-friendly); row 48/112 = ones
    (bias folded into the contraction as K=49).
  - W loaded naturally [128ch, (o,t)], cast to bf16 with o-stride 49
    (bias appended per o), PE-transposed per-o into W^T [49, 128ch]
    at row bases 0 (j0) / 64 (j1) via tile_position col packing.
  - Per-channel matmul: lhsT = x^T_aug [49,128b] (stationary, bf16),
    rhs = W^T_aug [49,48o], out psum [128b, 48o] fp32.
  - out stored naturally [b=128 partitions, (o, g)] at full DMA width.
"""

import numpy as np
import ml_dtypes

import concourse.bass as bass  # noqa: F401
import concourse.mybir as mybir
import concourse.tile as tile
from concourse import bacc
from concourse.bass_utils import run_bass_kernel_spmd

B, T, O, N = 128, 48, 48, 64
C = N * N
NCORES = 8
CS = C // NCORES  # 512 channels per core
KAUG = T + 1  # 49: contraction rows = 48 t's + 1 bias row
GH = 256  # channels per j-half
NG = CS // (2 * GH)  # 1 group of 512 channels
BC = 16  # b-chunk for x staging DMA
NBC = B // BC

F32 = mybir.dt.float32
BF16 = mybir.dt.bfloat16


def _body(tc, nc, x_d, w_d, b_d, out_d, ident_d, ones_d):
    PS = 8  # channels per psum tile (4 per j-half per bank)
    BQ = 32  # b-quarter for out tiles
    NBQ = B // BQ
    with (
        tc.tile_pool(name="const", bufs=1) as cpool,
        tc.tile_pool(name="xbf", bufs=1) as xb_pool,
        tc.tile_pool(name="wbf", bufs=4) as wb_pool,
        tc.tile_pool(name="wt", bufs=1) as wt_pool,
        tc.tile_pool(name="outs", bufs=5) as os_pool,
        tc.tile_pool(name="tpsum", bufs=3, space="PSUM") as tp_pool,
        tc.tile_pool(name="mpsum", bufs=2, space="PSUM") as mp_pool,
    ):
        idt = cpool.tile([128, 128], BF16)
        nc.sync.dma_start(idt[:, :], ident_d)

        # ---- loads (SWDGE FIFO order: W+bias, x, ones) ----
        # WT rows: {0-47: t j0, 48: bias j0, 64-111: t j1, 112: bias j1}
        # +16 pad cols so M=64 matmuls can over-read past the last channel
        wt = wt_pool.tile([128, GH * O + 16], BF16)  # col = g*O + o
        nc.vector.memset(wt[:, GH * O : GH * O + 16], 0.0)
        wbfs = {}
        for j in range(2):
            for gh in range(2):
                g0 = j * GH + gh * 128
                wbf = wb_pool.tile([128, O * T], BF16)
                nc.gpsimd.dma_start(
                    wbf[:, :], w_d[g0 : g0 + 128].rearrange("g o t -> g (o t)")
                )
                wbfs[(j, gh)] = wbf
            nc.gpsimd.dma_start(
                wt[j * 64 + T : j * 64 + T + 1, 0 : GH * O],
                b_d[j * GH : (j + 1) * GH].rearrange("g o -> (g o)").unsqueeze(0),
            )
        xbf = xb_pool.tile([128, B * GH], BF16)  # col = b*GH + g
        for bc in range(NBC):
            for j in range(2):
                src_ = x_d[
                    bc * BC : (bc + 1) * BC, :, j * GH : (j + 1) * GH
                ].rearrange("b t g -> t b g")
                dst = xbf[
                    j * 64 : j * 64 + T, bc * BC * GH : (bc + 1) * BC * GH
                ].rearrange("t (b g) -> t b g", g=GH)
                nc.gpsimd.dma_start(dst, src_)
        for j in range(2):
            nc.gpsimd.dma_start(
                xbf[j * 64 + T : j * 64 + T + 1, :], ones_d[j : j + 1, :]
            )

        # ---- W transposes into W^T ----
        for gh in range(2):
            gof = gh * 128 * O
            wt3 = wt[:, gof : gof + 128 * O].rearrange("t (g o) -> t o g", o=O)
            for oq in range(O // 4):
                pt = tp_pool.tile([128, 512], BF16)
                for os_ in range(4):
                    o = oq * 4 + os_
                    csl = slice(os_ * 128, (os_ + 1) * 128)
                    nc.tensor.transpose(
                        pt[0:T, csl], wbfs[(0, gh)][:, o * T : (o + 1) * T], idt[:, :]
                    )
                    nc.tensor.transpose(
                        pt[64 : 64 + T, csl],
                        wbfs[(1, gh)][:, o * T : (o + 1) * T],
                        idt[:, :],
                    )
                pt3 = pt[:, :].rearrange("p (o g) -> p o g", g=128)
                osl = slice(oq * 4, (oq + 1) * 4)
                if oq % 2 == 0:
                    nc.vector.tensor_copy(wt3[0:T, osl, :], pt3[0:T])
                    nc.scalar.copy(wt3[64 : 64 + T, osl, :], pt3[64 : 64 + T])
                else:
                    nc.scalar.copy(wt3[0:T, osl, :], pt3[0:T])
                    nc.vector.tensor_copy(wt3[64 : 64 + T, osl, :], pt3[64 : 64 + T])

        # ---- matmuls (out^T = W_c @ X_c^T, j-paired rows) + stores ----
        # outs tiles keyed (bq, ghalf); ghalf 0 completes at pg 15 so its
        # stores overlap the second half's matmuls.
        outs_raw = {}
        outs_tiles = {}
        xbf3 = xbf[:, :].rearrange("t (b g) -> t b g", g=GH)
        for pg in range(GH // PS):
            gh, pgh = divmod(pg, 16)
            if pgh == 0:
                for bq in range(NBQ):
                    outs = os_pool.tile([128, BQ * 128], F32)  # col = b*128+g
                    outs_raw[(bq, gh)] = outs
                    outs_tiles[(bq, gh)] = outs[:, :].rearrange(
                        "r (b p h k) -> r p h b k", p=16, h=2, k=4
                    )
            # psum col = h*512 + b*4 + kk (h = bank half, k = h*4 + kk)
            pt = mp_pool.tile([128, B * PS], F32)
            pt4 = pt[:, :].rearrange("r (h b k) -> r h b k", h=2, k=4)
            for k in range(PS):
                g = pg * PS + k
                h, kk = divmod(k, 4)
                for j in range(2):
                    r0 = j * 64
                    nc.tensor.matmul(
                        pt4[r0 : r0 + 64, h, :, kk : kk + 1],
                        lhsT=wt[r0 : r0 + KAUG, g * O : g * O + 64],
                        rhs=xbf3[r0 : r0 + KAUG, :, g : g + 1],
                        start=(kk == 0),
                        stop=(kk == 3),
                        skip_group_check=True,
                    )
            for bq in range(NBQ):
                src = pt4[:, :, bq * BQ : (bq + 1) * BQ, :]
                dst = outs_tiles[(bq, gh)][:, pgh, :, :, :]
                if (pg + bq) % 2 == 0:
                    nc.vector.tensor_copy(dst, src)
                else:
                    nc.scalar.copy(dst, src)
            if pgh == 15:
                for bq in range(NBQ):
                    for j in range(2):
                        c0 = j * GH + gh * 128
                        dst = out_d[
                            bq * BQ : (bq + 1) * BQ, :, c0 : c0 + 128
                        ].rearrange("b o g -> o b g")
                        src_ = outs_raw[(bq, gh)][j * 64 : j * 64 + O, :].rearrange(
                            "r (b g) -> r b g", g=128
                        )
                        eng = (nc.sync, nc.scalar, nc.gpsimd)[(bq * 2 + j) % 3]
                        eng.dma_start(dst, src_)


def build_program(num_devices=NCORES):
    nc = bacc.Bacc(
        "TRN2",
        target_bir_lowering=False,
        debug=False,
        enable_asserts=False,
        num_devices=num_devices,
    )
    x_d = nc.dram_tensor("x", [B, T, CS], F32, kind="ExternalInput").ap()
    w_d = nc.dram_tensor("w", [CS, O, T], F32, kind="ExternalInput").ap()
    b_d = nc.dram_tensor("bias", [CS, O], F32, kind="ExternalInput").ap()
    out_d = nc.dram_tensor("out", [B, T, CS], F32, kind="ExternalOutput").ap()
    ident_d = nc.inline_tensor(
        np.eye(128, dtype=ml_dtypes.bfloat16), name="identc"
    ).ap()
    ones_d = nc.inline_tensor(
        np.ones([2, GH * B], dtype=ml_dtypes.bfloat16), name="onesc"
    ).ap()
    with tile.TileContext(nc) as tc:
        _body(tc, nc, x_d, w_d, b_d, out_d, ident_d, ones_d)
    nc.compile()
    return nc


_CACHED_NC = None
LAST_RESULT = None


def kernel(**inputs) -> np.ndarray:
    global _CACHED_NC, LAST_RESULT
    x = np.ascontiguousarray(np.asarray(inputs["x"], dtype=np.float32)).reshape(
        B, T, C
    )
    W = np.ascontiguousarray(np.asarray(inputs["W"], dtype=np.float32))
    bias = np.ascontiguousarray(np.asarray(inputs["b"], dtype=np.float32))

    if _CACHED_NC is None:
        _CACHED_NC = build_program(NCORES)
    nc = _CACHED_NC

    in_maps = []
    for i in range(NCORES):
        sl = slice(i * CS, (i + 1) * CS)
        in_maps.append(
            {
                "x": np.ascontiguousarray(x[:, :, sl]),
                "w": np.ascontiguousarray(W[sl]),
                "bias": np.ascontiguousarray(bias[sl]),
            }
        )
    res = run_bass_kernel_spmd(nc, in_maps, core_ids=list(range(NCORES)))
    LAST_RESULT = res
    out = np.concatenate([res.results[i]["out"] for i in range(NCORES)], axis=2)
    return out.reshape(B, T, N, N)



# revision 2
# speedup vs baseline: 2.9234x; 2.9234x over previous
"""Per-channel batched Linear (OD matrix) Trainium2 Bass kernel, v2.

Computes out[b,o,c] = sum_t x[b,t,c] * W[c,o,t] + bias[c,o] for
x [128,48,4096], W [4096,48,48], bias [4096,48].

Strategy (8 NeuronCores, channel-parallel, 512 channels/core):
  - Host pre-packs per core (numpy, not timed by the HW profile):
      XA [98, 256, 128] bf16: rows 0-47 = x^T[t, c_lo, b], row 48 = ones,
        rows 49-96 = x^T[t, c_hi, b], row 97 = ones  (c_lo = s, c_hi = 256+s)
      WA [98, 256, 96] bf16 block-diagonal per slot s:
        rows 0-47 cols 0-47 = W[c_lo]^T, row 48 cols 0-47 = bias[c_lo],
        rows 49-96 cols 48-95 = W[c_hi]^T, row 97 cols 48-95 = bias[c_hi],
        zeros elsewhere.
  - One matmul per slot: lhsT = WA[:, s, :] [K=98, M=96] (block-diag pair),
    rhs = XA[:, s, :] [98, 128 b] -> psum [96, 128] = both channels'
    out[o, b] stacked (rows 0-47 c_lo, rows 48-95 c_hi). Bias rides as
    K rows 48/97 against the ones rows of XA.
  - 4 slots per psum bank; one contiguous [96, 512] f32->bf16 copy per
    bank (DVE mostly, ACT every 4th) into staged tiles; 16KB/partition
    contiguous stores every 64 slots via SWDGE.
  - All DMA runs are >=12KB contiguous per partition; everything bf16.
  - Host un-packs out [48, 512, 128] -> [b, t, c] and casts to f32.
"""

import numpy as np
import ml_dtypes

import concourse.bass as bass  # noqa: F401
import concourse.mybir as mybir
import concourse.tile as tile
from concourse import bacc
from concourse.bass_utils import run_bass_kernel_spmd

B, T, O, N = 128, 48, 48, 64
C = N * N
NCORES = 8
CS = C // NCORES  # 512 channels per core
S = CS // 2  # 256 slots (channel pairs) per core
K = 2 * (T + 1)  # 98 contraction rows (2 x (48 t + bias row))
MS = 2 * O  # 96 psum partitions per slot (2 x 48 o)

F32 = mybir.dt.float32
BF16 = mybir.dt.bfloat16
BF16_NP = ml_dtypes.bfloat16


def _body(tc, nc, xa_d, wa_d, out_d):
    NCH = 4  # load chunks (64 slots each)
    SG = 64  # slots per staged/store group
    with (
        tc.tile_pool(name="xa", bufs=1) as xa_pool,
        tc.tile_pool(name="wa", bufs=1) as wa_pool,
        tc.tile_pool(name="stg", bufs=3) as stg_pool,
        tc.tile_pool(name="ps", bufs=8, space="PSUM") as ps_pool,
    ):
        xa = xa_pool.tile([K, S * B], BF16)
        wa = wa_pool.tile([K, S * MS], BF16)
        xa3 = xa[:, :].rearrange("k (s b) -> k s b", b=B)
        wa3 = wa[:, :].rearrange("k (s m) -> k s m", m=MS)
        for ch in range(NCH):
            s0, s1 = ch * (S // NCH), (ch + 1) * (S // NCH)
            nc.sync.dma_start(wa3[:, s0:s1, :], wa_d[:, s0:s1, :])
            nc.sync.dma_start(xa3[:, s0:s1, :], xa_d[:, s0:s1, :])

        pt = None
        stg = None
        for s in range(S):
            q = s % 4
            bank = s // 4
            bg = bank % (SG // 4)
            if s % SG == 0:
                stg = stg_pool.tile([MS, SG * B], BF16)
            if q == 0:
                pt = ps_pool.tile([MS, 4 * B], F32)
            nc.tensor.matmul(
                pt[:, q * B : (q + 1) * B],
                lhsT=wa3[:, s, :],
                rhs=xa3[:, s, :],
                start=True,
                stop=True,
            )
            if q == 3:
                dst = stg[:, bg * 4 * B : (bg + 1) * 4 * B]
                if bank % 4 == 3:
                    nc.scalar.copy(dst, pt[:, :])
                else:
                    nc.vector.tensor_copy(dst, pt[:, :])
            if s % SG == SG - 1:
                sg = s // SG
                for h in range(2):
                    dst = out_d[:, h * S + sg * SG : h * S + (sg + 1) * SG, :]
                    src = stg[h * O : (h + 1) * O, :].rearrange(
                        "o (s b) -> o s b", b=B
                    )
                    nc.gpsimd.dma_start(dst, src)


def build_program(num_devices=NCORES):
    nc = bacc.Bacc(
        "TRN2",
        target_bir_lowering=False,
        debug=False,
        enable_asserts=False,
        num_devices=num_devices,
    )
    xa_d = nc.dram_tensor("xa", [K, S, B], BF16, kind="ExternalInput").ap()
    wa_d = nc.dram_tensor("wa", [K, S, MS], BF16, kind="ExternalInput").ap()
    out_d = nc.dram_tensor("out", [O, CS, B], BF16, kind="ExternalOutput").ap()
    with tile.TileContext(nc) as tc:
        _body(tc, nc, xa_d, wa_d, out_d)
    nc.compile()
    return nc


_CACHED_NC = None
LAST_RESULT = None


def kernel(**inputs) -> np.ndarray:
    global _CACHED_NC, LAST_RESULT
    x = np.asarray(inputs["x"], dtype=np.float32).reshape(B, T, C)
    W = np.asarray(inputs["W"], dtype=np.float32)
    bias = np.asarray(inputs["b"], dtype=np.float32)

    xtb = x.transpose(1, 2, 0).astype(BF16_NP)  # [T, C, B]
    Wtb = W.transpose(2, 0, 1).astype(BF16_NP)  # [T, C, O]
    bb = bias.astype(BF16_NP)  # [C, O]

    if _CACHED_NC is None:
        _CACHED_NC = build_program(NCORES)
    nc = _CACHED_NC

    in_maps = []
    for i in range(NCORES):
        lo = i * CS
        XA = np.empty((K, S, B), BF16_NP)
        XA[:T] = xtb[:, lo : lo + S]
        XA[T] = 1.0
        XA[T + 1 : K - 1] = xtb[:, lo + S : lo + CS]
        XA[K - 1] = 1.0
        WA = np.zeros((K, S, MS), BF16_NP)
        WA[:T, :, :O] = Wtb[:, lo : lo + S]
        WA[T, :, :O] = bb[lo : lo + S]
        WA[T + 1 : K - 1, :, O:] = Wtb[:, lo + S : lo + CS]
        WA[K - 1, :, O:] = bb[lo + S : lo + CS]
        in_maps.append({"xa": XA, "wa": WA})
    res = run_bass_kernel_spmd(nc, in_maps, core_ids=list(range(NCORES)))
    LAST_RESULT = res
    # out [O, CS, B] per core -> [B, T, C]
    full = np.concatenate(
        [np.asarray(res.results[i]["out"]) for i in range(NCORES)], axis=1
    )
    out = full.transpose(2, 0, 1).astype(np.float32)
    return np.ascontiguousarray(out).reshape(B, T, N, N)
